# revision 1
# baseline (speedup 1.0000x reference)
"""Trainium2 Bass kernel for nn_LoopedMLP (moe_routing).

Reference semantics (B=8, T=1024, C=1024, ITER=4096, FULL=12288, R=0.7):
a 3-step scan; each step computes
    y = relu((x @ Wm^T) * active_mask) @ Wp^T
then a control net on mean-pooled y picks a new top-4096 column mask, and a
batch-mean "novelty" below R freezes the state for remaining steps.

Because relu((x@Wm^T)*mask) @ Wp^T only touches the masked 4096 columns, each
step is exactly a dense per-sample MLP over the *gathered* active columns:
    y = relu(x @ A^T) @ B,   A = Wm[idx], B = Wp[:, idx]^T,  |idx| = 4096.

Strategy: data-parallel over B (1 sample per NeuronCore, 8 cores). The device
kernel computes the gathered MLP; all routing logic (control net, top-k,
novelty, done) runs on host between launches. With random weights the step-1
novelty is ~2/3 < R, so in practice exactly one device launch happens.

Device compute per core: two back-to-back matmul chains in fp16 (stationary
weights, 1 moving column/cycle at N=512) with fp32 PSUM accumulation:
  phase 1: H^T[j,t]   = relu( sum_c A^T[c,j] * X^T[c,t] )  (j=4096, t=1024, c=1024)
  phase 2: Y^T[c,t]   = sum_j B[j,c] * H^T[j,t]
~1024 matmuls of [128x128]@[128x512] at ~216 ns warm => ~221 us of PE stream.
(fp8 DoubleRow would be 2x but its e6m3 operand upcast caps precision at 3
mantissa bits => >=4e-2 abs-max rel error per fp8 phase vs the 2e-2 gate,
verified numerically — fp16 is the fastest dtype that passes.)

Startup/tail engineering vs the 242.2 us baseline (all trace-driven;
measured best 237.9 us, stall-free stream, occasional chip-level P0 power
throttling can still stretch a run to ~285 us — outside kernel control):
 - input arrival curve: per-queue DMA sustains only ~90-135 GB/s
   (3 queues share ~270 GB/s of HBM), so the 2.5 MB the head needs defines
   a critical stream-start time S* ~= 12 us. x tiles ride scalar (0/1/3/5)
   and gpsimd (2/4); the sync queue carries at0, at1 then the two
   latest-needed x tiles (6/7) before the at stream — this balances the
   three queues and compresses S* by ~2 us vs a 2-queue x split.
 - 11 warmup matmuls on a zeroed tile keep the PE continuously busy from
   ~8 us so the HAM clock gate opens at ~11.5 us (any pre-flip idle gap
   restarts its 3.4 us activity window — measured) and the real stream
   starts at S* fully warm; the head (j-tiles 0/1, PSUM ct-accumulation
   commutes) is emitted in measured-arrival order, giving a zero-stall
   stream end to end.
 - the Tile scheduler orders each engine queue from its own cost model and
   otherwise hoists late work ahead of ready work (observed on 3 queues);
   monotone tc.tile_wait_until stamps pin every queue to the planned
   order, and the 8 MB b-tile prefetch is runtime-gated behind the x
   stream by a 1-element gpsimd copy that reads xts[7].
 - phase-2 final accumulation group (th=1, c8=7) split into 2x N=256
   groups; the first half's copy+DMA hides under the second half's
   matmuls and the last half is emitted fp16 (64KB -> 32KB) to shrink the
   exposed tail (measured: tail after last MM dropped 5.8 us -> 1.2 us).
"""

import os
import sys

import numpy as np


def _ensure_concourse():
    try:
        import concourse  # noqa: F401
    except ImportError:
        for p in ("/opt/trn_rl_repo", "/root/.axon_site/_ro/trn_rl_repo"):
            if os.path.isdir(p) and p not in sys.path:
                sys.path.insert(0, p)
        import concourse  # noqa: F401


N_EMBD = 1024
T_SEQ = 1024
ITER = 4096
FULL = 12288
R_NOVELTY = 0.7
NCORES = 8
JT = ITER // 128   # 32 j-tiles
CT = N_EMBD // 128  # 8 c-tiles
HEAD_JT = 2        # j-tiles computed ct-outer while the x stream lands
N_WARM = 12        # 8 cold + 4 warm matmuls: continuous PE busy from ~8.0us
                   # flips the HAM clock gate at ~11.4us and lands exactly at
                   # the critical stream-start time S* ~= 12.0us, after which
                   # the measured input-arrival curve stays ahead of the
                   # 433ns/pair consumption with zero stalls (a stall before
                   # the flip restarts the 3.4us activity window — measured).

# head emission order (jt, ct): greedy vs measured arrivals for the queue
# assignment below (xt0,1,3,5 on scalar; xt2,4 on gpsimd; sync runs
# at0, at1, xt6, xt7, then the at stream).
_HEAD_ORDER = [(0, 0), (0, 2), (1, 0), (1, 2), (0, 1), (1, 1),
               (0, 4), (1, 4), (0, 6), (1, 6), (0, 3), (1, 3),
               (0, 7), (1, 7), (0, 5), (1, 5)]

_STATE: dict = {}


# ---------------------------------------------------------------- device side

def _build_nc():
    _ensure_concourse()
    import concourse.tile as tile
    from concourse import bacc, mybir
    from concourse.bass import ts

    f32 = mybir.dt.float32
    f16 = mybir.dt.float16
    relu = mybir.ActivationFunctionType.Relu

    nc = bacc.Bacc("TRN2", target_bir_lowering=False, debug=False,
                   num_devices=NCORES)
    xa = nc.dram_tensor("xt", [CT, 128, T_SEQ], f16, kind="ExternalInput").ap()
    aa = nc.dram_tensor("at", [JT, 128, N_EMBD], f16, kind="ExternalInput").ap()
    ba = nc.dram_tensor("bt", [8, 128, 4 * N_EMBD], f16,
                        kind="ExternalInput").ap()
    # output is Y^T tiles: [t-half, c-tile, 128 c, 512 t]; the final half
    # group (th=1, c8=7, t 256:512) is emitted separately in fp16.
    ya = nc.dram_tensor("y", [2, 8, 128, 512], f32, kind="ExternalOutput").ap()
    y2a = nc.dram_tensor("y2", [128, 256], f16, kind="ExternalOutput").ap()

    with tile.TileContext(nc) as tc:
        with (
            tc.tile_pool(name="xt", bufs=CT) as xt_pool,
            tc.tile_pool(name="wm", bufs=1) as wm_pool,
            tc.tile_pool(name="ht", bufs=JT) as ht_pool,
            tc.tile_pool(name="at", bufs=6) as at_pool,
            tc.tile_pool(name="bt", bufs=8) as bt_pool,
            tc.tile_pool(name="yo", bufs=4) as yo_pool,
            tc.tile_pool(name="yz", bufs=2) as yz_pool,
            tc.tile_pool(name="ps", bufs=8, space="PSUM") as ps_pool,
        ):
            # A-tile stream all on the sync HWDGE queue (sustains ~135 GB/s,
            # ahead of the 3.46us/tile consumption); at0/at1 first.
            att = {}

            def at_dma(jt):
                t = at_pool.tile([128, N_EMBD], f16, tag="at", name=f"a{jt}")
                nc.sync.dma_start(out=t[:], in_=aa[jt])
                att[jt] = t

            at_dma(0)
            at_dma(1)

            # x tiles [128, 1024]: 0/1/3/5 on the scalar HWDGE queue, 2/4 on
            # gpsimd, 6/7 as the sync queue's 3rd/4th transfers (the sync
            # queue idles after at0/at1 — carrying the two latest-needed x
            # tiles there compresses the arrival curve by ~2us).
            xts = [xt_pool.tile([128, T_SEQ], f16, tag="xt", name=f"x{ct}")
                   for ct in range(CT)]
            for ct in (0, 1, 3, 5):
                nc.scalar.dma_start(out=xts[ct][:], in_=xa[ct])
            for ct in (2, 4):
                nc.gpsimd.dma_start(out=xts[ct][:], in_=xa[ct])
            nc.sync.dma_start(out=xts[6][:], in_=xa[6])
            nc.sync.dma_start(out=xts[7][:], in_=xa[7])

            # at2 rides the sync queue right after xt6/xt7.
            with tc.tile_wait_until(0.0145):
                at_dma(2)

            # PE warmup on a zeroed tile: continuous busy through the input
            # DMA wait so the HAM clock gate opens before the real stream.
            wt = wm_pool.tile([128, 512], f16, tag="warm", name="warm")
            nc.vector.memset(wt[:], 0)
            wps = ps_pool.tile([128, 512], f32, tag="ps", name="wps")
            for i in range(N_WARM):
                nc.tensor.matmul(wps[:], lhsT=wt[:, ts(0, 128)], rhs=wt[:],
                                 start=True, stop=True)

            # phase 1 head: j-tiles 0/1 emitted in input-arrival order
            # (PSUM accumulation over ct commutes; start/stop per first/last
            # emitted ct of each (jt, th) group). Monotone wait stamps pin
            # the Tile scheduler to this order (its own DMA-arrival model
            # otherwise reorders the stream and hoists late-arriving work
            # ahead of ready work).
            head_ps = [[ps_pool.tile([128, 512], f32, tag="ps",
                                     name=f"hps{jt}_{i}") for i in range(2)]
                       for jt in range(HEAD_JT)]
            seen = [0] * HEAD_JT
            head_end = 0.0190
            for k, (jt, ct) in enumerate(_HEAD_ORDER):
                seen[jt] += 1
                with tc.tile_wait_until(0.0120 + 0.00043 * k):
                    for th in range(2):
                        nc.tensor.matmul(
                            head_ps[jt][th][:], lhsT=att[jt][:, ts(ct, 128)],
                            rhs=xts[ct][:, ts(th, 512)],
                            start=(seen[jt] == 1), stop=(seen[jt] == CT))
            hts = []
            for jt in range(HEAD_JT):
                htt = ht_pool.tile([128, T_SEQ], f16, tag="ht", name=f"h{jt}")
                with tc.tile_wait_until(head_end):
                    for th in range(2):
                        nc.scalar.activation(htt[:, ts(th, 512)],
                                             head_ps[jt][th][:], relu)
                hts.append(htt)

            # phase 1 main loop: stationary A-tile per jt, 16 matmuls.
            # Same wait-stamp trick: consumption-schedule stamps keep the
            # at-DMA stream behind xt7 on the sync queue and stop main-loop
            # matmuls from hoisting into the head.
            for jt in range(HEAD_JT, JT):
                with tc.tile_wait_until(
                        head_end + 0.00346 * (jt - HEAD_JT)):
                    at_dma(jt)
                    a = att[jt]
                    ps = [ps_pool.tile([128, 512], f32, tag="ps",
                                       name=f"ps{jt}_{i}") for i in range(2)]
                    # th-outer: consecutive matmuls accumulate into the SAME
                    # PSUM bank (like phase 2, which measures 512 cyc/MM flat
                    # vs 519.6 for the bank-alternating th-inner order).
                    for th in range(2):
                        for ct in range(CT):
                            nc.tensor.matmul(
                                ps[th][:], lhsT=a[:, ts(ct, 128)],
                                rhs=xts[ct][:, ts(th, 512)],
                                start=(ct == 0), stop=(ct == CT - 1))
                    htt = ht_pool.tile([128, T_SEQ], f16, tag="ht",
                                       name=f"h{jt}")
                    for th in range(2):
                        nc.scalar.activation(htt[:, ts(th, 512)], ps[th][:],
                                             relu)
                    hts.append(htt)

            # B tiles: 1MB DMAs on gpsimd only, runtime-gated behind the x
            # stream. The 1-element gpsimd copy reading xts[7] blocks the
            # gpsimd engine until the last x tile has landed, so the bt
            # transfers cannot eat HBM bandwidth during the head (observed:
            # ungated bt transfers delayed xt5/xt7 to 23/28us and the head
            # stalls re-throttled the PE clock for 13.7us). tile_wait_until
            # additionally stops the scheduler from hoisting them.
            bts = []
            for g in range(8):
                btt = bt_pool.tile([128, 4 * N_EMBD], f16, tag="bt",
                                   name=f"b{g}")
                with tc.tile_wait_until(0.020 + 0.005 * g):
                    nc.gpsimd.tensor_copy(btt[:, 0:1], xts[7][:, 0:1])
                    nc.gpsimd.dma_start(out=btt[:], in_=ba[g])
                bts.append(btt)

            def bt_ap(jt, c8):
                return bts[jt // 4][:, (jt % 4) * N_EMBD + 128 * c8:
                                    (jt % 4) * N_EMBD + 128 * (c8 + 1)]

            # phase 2: Y^T[c,t] accumulated over j. c8-outer/jt-inner so all
            # copies/output DMAs except the last hide under the MM stream.
            for th in range(2):
                for c8 in range(8):
                    if th == 1 and c8 == 7:
                        break
                    pst = ps_pool.tile([128, 512], f32, tag="ps",
                                       name=f"yps{th}_{c8}")
                    for jt in range(JT):
                        nc.tensor.matmul(
                            pst[:], lhsT=bt_ap(jt, c8),
                            rhs=hts[jt][:, ts(th, 512)],
                            start=(jt == 0), stop=(jt == JT - 1))
                    yo = yo_pool.tile([128, 512], f32, tag="yo",
                                      name=f"y{th}_{c8}")
                    if c8 % 2 == 0:
                        nc.vector.tensor_copy(yo[:], pst[:])
                        nc.sync.dma_start(out=ya[th, c8], in_=yo[:])
                    else:
                        nc.scalar.copy(yo[:], pst[:])
                        nc.scalar.dma_start(out=ya[th, c8], in_=yo[:])

            # final group (th=1, c8=7) split into 2x N=256 so the exposed
            # tail is one small fp16 transfer instead of a 256KB fp32 one.
            psA = ps_pool.tile([128, 256], f32, tag="ps", name="ypsA")
            for jt in range(JT):
                nc.tensor.matmul(psA[:], lhsT=bt_ap(jt, 7),
                                 rhs=hts[jt][:, 512:768],
                                 start=(jt == 0), stop=(jt == JT - 1))
            yoA = yz_pool.tile([128, 256], f32, tag="yz", name="yA")
            nc.vector.tensor_copy(yoA[:], psA[:])
            nc.sync.dma_start(out=ya[1, 7, :, 0:256], in_=yoA[:])

            psB = ps_pool.tile([128, 256], f32, tag="ps", name="ypsB")
            for jt in range(JT):
                nc.tensor.matmul(psB[:], lhsT=bt_ap(jt, 7),
                                 rhs=hts[jt][:, 768:1024],
                                 start=(jt == 0), stop=(jt == JT - 1))
            yoB = yz_pool.tile([128, 256], f16, tag="yz", name="yB")
            # DVE copy: 2x rate at 16-bit output (~200ns vs scalar's 473ns)
            # on the only copy whose latency is exposed in the tail.
            nc.vector.tensor_copy(yoB[:], psB[:])
            nc.scalar.dma_start(out=y2a[:], in_=yoB[:])

    nc.compile()
    return nc


class _Runner:
    """Persistent jitted SPMD dispatcher (mirrors bass2jax.run_bass_via_pjrt's
    multi-core branch, but reuses one jax.jit across calls)."""

    def __init__(self, nc):
        _ensure_concourse()
        import jax
        import concourse.mybir as mybir
        from concourse import bass2jax
        from jax.experimental.shard_map import shard_map
        from jax.sharding import Mesh, PartitionSpec

        bass2jax.install_neuronx_cc_hook()
        self.nc = nc
        partition_name = (nc.partition_id_tensor.name
                          if nc.partition_id_tensor else None)
        in_names, out_names, out_avals, zero_shapes = [], [], [], []
        for alloc in nc.m.functions[0].allocations:
            if not isinstance(alloc, mybir.MemoryLocationSet):
                continue
            name = alloc.memorylocations[0].name
            if alloc.kind == "ExternalInput":
                if name != partition_name:
                    in_names.append(name)
            elif alloc.kind == "ExternalOutput":
                shape = tuple(alloc.tensor_shape)
                dtype = mybir.dt.np(alloc.dtype)
                out_names.append(name)
                out_avals.append(jax.core.ShapedArray(shape, dtype))
                zero_shapes.append((shape, dtype))
        self.in_names = list(in_names)
        self.out_names = out_names
        self.out_avals = out_avals
        self.zero_shapes = zero_shapes
        n_params = len(in_names)
        all_in_names = in_names + out_names
        if partition_name is not None:
            all_in_names.append(partition_name)

        def _body(*args):
            operands = list(args)
            if partition_name is not None:
                operands.append(bass2jax.partition_id_tensor())
            outs = bass2jax._bass_exec_p.bind(
                *operands,
                out_avals=tuple(out_avals),
                in_names=tuple(all_in_names),
                out_names=tuple(out_names),
                lowering_input_output_aliases=(),
                sim_require_finite=True,
                sim_require_nnan=True,
                nc=nc,
            )
            return tuple(outs)

        devices = jax.devices()[:NCORES]
        assert len(devices) == NCORES
        self.mesh = Mesh(np.asarray(devices), ("core",))
        n_outs = len(out_names)
        in_specs = (PartitionSpec("core"),) * (n_params + n_outs)
        out_specs = (PartitionSpec("core"),) * n_outs
        self.donate = tuple(range(n_params, n_params + n_outs))
        self.fn = jax.jit(
            shard_map(_body, mesh=self.mesh, in_specs=in_specs,
                      out_specs=out_specs, check_rep=False),
            donate_argnums=self.donate, keep_unused=True)

    def concat_inputs(self, in_maps):
        return [np.concatenate([np.asarray(m[n]) for m in in_maps], axis=0)
                for n in self.in_names]

    def zero_outs(self):
        return [np.zeros((NCORES * s[0], *s[1:]), d)
                for (s, d) in self.zero_shapes]

    def __call__(self, in_maps):
        concat_in = self.concat_inputs(in_maps)
        out_arrs = self.fn(*concat_in, *self.zero_outs())
        return [
            {n: np.asarray(out_arrs[i]).reshape(NCORES, *self.out_avals[i].shape)[c]
             for i, n in enumerate(self.out_names)}
            for c in range(NCORES)
        ]


def _get_runner():
    if "runner" not in _STATE:
        nc = _build_nc()
        _STATE["nc"] = nc
        _STATE["runner"] = _Runner(nc)
    return _STATE["runner"]


# ------------------------------------------------------------------ host side

def _tile_A(A):
    """(4096, 1024) row-gathered Wm -> fp16 'at' tiles [jt, ci, ct*128+jj]."""
    return np.ascontiguousarray(
        A.reshape(JT, 128, CT, 128).transpose(0, 3, 2, 1)).reshape(
            JT, 128, N_EMBD).astype(np.float16)


def _tile_B(Bm):
    """(4096, 1024) row-gathered Wp^T -> fp16 'bt' layout [8, jj, 4jt*c]."""
    t = np.ascontiguousarray(Bm).reshape(8, 4, 128, N_EMBD).astype(np.float16)
    return np.ascontiguousarray(t.transpose(0, 2, 1, 3)).reshape(
        8, 128, 4 * N_EMBD)


def _tile_X(xc):
    """(B, 1024 t, 1024 c) -> per-core fp16 xt tiles [B, ct, ci, t]."""
    return np.ascontiguousarray(xc.transpose(0, 2, 1)).reshape(
        xc.shape[0], CT, 128, T_SEQ).astype(np.float16)


def _untile_Y(res):
    """{'y': [th,c8,ci,tt] f32, 'y2': [ci,256] f16} -> f32 (1024 t, 1024 c)."""
    y = np.ascontiguousarray(
        res["y"].transpose(0, 3, 1, 2)).reshape(T_SEQ, N_EMBD).astype(
            np.float32, copy=False)
    y[768:1024, 896:1024] = res["y2"].T.astype(np.float32)
    return y


def _device_forward(xc, at_list, bt_list):
    """y[b] = relu(xc[b] @ A^T) @ B for 8 cores at once."""
    xts = _tile_X(xc)
    in_maps = []
    for b in range(NCORES):
        in_maps.append({"xt": xts[b], "at": at_list[b], "bt": bt_list[b]})
    try:
        results = _get_runner()(in_maps)
    except Exception:
        # fall back to the supported dispatch path (fresh jit per call)
        from concourse.bass_utils import run_bass_kernel_spmd
        if "nc" not in _STATE:
            _STATE["nc"] = _build_nc()
        results = run_bass_kernel_spmd(
            _STATE["nc"], in_maps, list(range(NCORES))).results
    return np.stack([_untile_Y(results[b]) for b in range(NCORES)])


def _topk_mask(ck, k):
    # matches jax.lax.top_k tie-breaking (first index wins) via stable argsort
    order = np.argsort(-ck, axis=1, kind="stable")[:, :k]
    mask = np.zeros_like(ck)
    np.put_along_axis(mask, order, 1.0, axis=1)
    return mask


def kernel(x, Wm, Wp, Wc1, Wc2):
    x = np.ascontiguousarray(np.asarray(x, dtype=np.float32))
    Wm = np.ascontiguousarray(np.asarray(Wm, dtype=np.float32))
    Wp = np.ascontiguousarray(np.asarray(Wp, dtype=np.float32))
    Wc1 = np.asarray(Wc1, dtype=np.float32)
    Wc2 = np.asarray(Wc2, dtype=np.float32)
    B = x.shape[0]
    assert B == NCORES and x.shape[1] == T_SEQ and x.shape[2] == N_EMBD

    WpT = None  # lazily built; only needed on non-base iterations
    base = np.zeros((B, FULL), np.float32)
    base[:, :ITER] = 1.0

    xc, active, history, done = x, base, base.copy(), False
    for _ in range(3):
        if done:
            break
        idxs = [np.flatnonzero(active[b]) for b in range(B)]
        is_base = all(ix.shape[0] == ITER and ix[0] == 0 and ix[-1] == ITER - 1
                      for ix in idxs) and all(
                          np.array_equal(ix, idxs[0]) for ix in idxs[1:])
        if is_base and np.array_equal(idxs[0], np.arange(ITER)):
            at = _tile_A(Wm[:ITER])
            bt = _tile_B(np.ascontiguousarray(Wp[:, :ITER].T))
            at_list = [at] * B
            bt_list = [bt] * B
        else:
            if WpT is None:
                WpT = np.ascontiguousarray(Wp.T)
            at_list = [_tile_A(np.ascontiguousarray(Wm[ix])) for ix in idxs]
            bt_list = [_tile_B(WpT[ix]) for ix in idxs]

        y = _device_forward(xc, at_list, bt_list)

        pooled = y.mean(axis=1)
        ck = np.maximum(pooled @ Wc1.T, 0.0) @ Wc2.T
        new_mask = _topk_mask(ck, ITER)
        combined = np.clip(history + new_mask, 0.0, 1.0)
        novelty = (combined - history).sum(axis=1).mean() / ITER
        xc, active, history = y, new_mask, combined
        done = bool(novelty < R_NOVELTY)

    return xc.astype(np.float32, copy=False)



# revision 3
# speedup vs baseline: 1.0126x; 1.0126x over previous
"""Trainium2 Bass kernel for nn_LoopedMLP (moe_routing).

Reference semantics (B=8, T=1024, C=1024, ITER=4096, FULL=12288, R=0.7):
a 3-step scan; each step computes
    y = relu((x @ Wm^T) * active_mask) @ Wp^T
then a control net on mean-pooled y picks a new top-4096 column mask, and a
batch-mean "novelty" below R freezes the state for remaining steps.

Because relu((x@Wm^T)*mask) @ Wp^T only touches the masked 4096 columns, each
step is exactly a dense per-sample MLP over the *gathered* active columns:
    y = relu(x @ A^T) @ B,   A = Wm[idx], B = Wp[:, idx]^T,  |idx| = 4096.

Strategy: data-parallel over B (1 sample per NeuronCore, 8 cores). The device
kernel computes the gathered MLP; all routing logic (control net, top-k,
novelty, done) runs on host between launches. With random weights the step-1
novelty is ~2/3 < R, so in practice exactly one device launch happens.

Device compute per core: two back-to-back matmul chains in fp16 (stationary
weights, 1 moving column/cycle at N=512) with fp32 PSUM accumulation:
  phase 1: H^T[j,t]   = relu( sum_c A^T[c,j] * X^T[c,t] )  (j=4096, t=1024, c=1024)
  phase 2: Y^T[c,t]   = sum_j B[j,c] * H^T[j,t]
~1024 matmuls of [128x128]@[128x512] at ~216 ns warm => ~221 us of PE stream.
(fp8 DoubleRow would be 2x but its e6m3 operand upcast caps precision at 3
mantissa bits => >=4e-2 abs-max rel error per fp8 phase vs the 2e-2 gate,
verified numerically — fp16 is the fastest dtype that passes.)

Startup/tail engineering vs the 242.2 us baseline (all trace-driven;
measured best 237.9 us, stall-free stream, occasional chip-level P0 power
throttling can still stretch a run to ~285 us — outside kernel control):
 - input arrival curve: per-queue DMA sustains only ~90-135 GB/s
   (3 queues share ~270 GB/s of HBM), so the 2.5 MB the head needs defines
   a critical stream-start time S* ~= 12 us. x tiles ride scalar (0/1/3/5)
   and gpsimd (2/4); the sync queue carries at0, at1 then the two
   latest-needed x tiles (6/7) before the at stream — this balances the
   three queues and compresses S* by ~2 us vs a 2-queue x split.
 - 11 warmup matmuls on a zeroed tile keep the PE continuously busy from
   ~8 us so the HAM clock gate opens at ~11.5 us (any pre-flip idle gap
   restarts its 3.4 us activity window — measured) and the real stream
   starts at S* fully warm; the head (j-tiles 0/1, PSUM ct-accumulation
   commutes) is emitted in measured-arrival order, giving a zero-stall
   stream end to end.
 - the Tile scheduler orders each engine queue from its own cost model and
   otherwise hoists late work ahead of ready work (observed on 3 queues);
   monotone tc.tile_wait_until stamps pin every queue to the planned
   order, and the 8 MB b-tile prefetch is runtime-gated behind the x
   stream by a 1-element gpsimd copy that reads xts[7].
 - phase-2 final accumulation group (th=1, c8=7) split into N=256 +
   2x N=128 groups; every copy/DMA except the last 32KB fp16 one hides
   under the following sub-group's matmuls, minimizing the exposed tail.

Session-2 trace findings (exec ~238.3us fresh-run, +-1.2us thermal drift):
 - the stream is at its floor: ~8.0us fixed preamble (3.3us engine-init
   sem wait + ~1.25us iram TENSOR_LOAD + barriers), 12 warmups bridging
   to the x0 arrival (visible ~12.0us, first real MM 12.08 -- zero slack),
   then a ZERO-GAP TensorMatrix stream to ~233.6us, ~4.7us fixed tail
   (cast+DMA chain ~1.3 + completion sems ~0.9 + teardown ~2.5).
 - the arrival window is DMA-parallelism-bound at ~280GB/s aggregate
   (per-queue ~60-90GB/s, descriptor-latency-limited). Removing the
   duplicate at2 DMA (a jt==2 re-issue in the main loop) freed 256KB;
   the scheduler backfills with the at stream, so last-x stays ~17.0us.
 - measured dead ends: explicitly throttling the at stream behind x
   (FEWER active queues => aggregate drops to ~236GB/s, exec +4us);
   splitting first/last tiles into 128KB halves (descriptor-bound: a
   half takes ~2.6-4.3us vs 3.2 for a full tile, and a real MM that
   stalls pre-HAM-flip restarts the 3.4us clock-gate window, exec +1.2);
   hw-loop compression of phase 2 (branch overhead inside the dense MM
   stream, not attempted); fp8 (e6m3 upcast, fails the 2e-2 gate).
"""

import os
import sys

import numpy as np


def _ensure_concourse():
    try:
        import concourse  # noqa: F401
    except ImportError:
        for p in ("/opt/trn_rl_repo", "/root/.axon_site/_ro/trn_rl_repo"):
            if os.path.isdir(p) and p not in sys.path:
                sys.path.insert(0, p)
        import concourse  # noqa: F401


N_EMBD = 1024
T_SEQ = 1024
ITER = 4096
FULL = 12288
R_NOVELTY = 0.7
NCORES = 8
JT = ITER // 128   # 32 j-tiles
CT = N_EMBD // 128  # 8 c-tiles
HEAD_JT = 2        # j-tiles computed ct-outer while the x stream lands
N_WARM = 12        # 8 cold + 4 warm matmuls: continuous PE busy from ~8.0us
                   # flips the HAM clock gate at ~11.4us and lands exactly at
                   # the critical stream-start time S* ~= 12.0us, after which
                   # the measured input-arrival curve stays ahead of the
                   # 433ns/pair consumption with zero stalls (a stall before
                   # the flip restarts the 3.4us activity window — measured).

# head emission order (jt, ct): greedy vs measured arrivals for the queue
# assignment below (xt0,1,3,5 on scalar; xt2,4 on gpsimd; sync runs
# at0, at1, xt6, xt7, then the at stream).
_HEAD_ORDER = [(0, 0), (0, 2), (1, 0), (1, 2), (0, 1), (1, 1),
               (0, 4), (1, 4), (0, 6), (1, 6), (0, 3), (1, 3),
               (0, 7), (1, 7), (0, 5), (1, 5)]

_STATE: dict = {}


# ---------------------------------------------------------------- device side

def _build_nc():
    _ensure_concourse()
    import concourse.tile as tile
    from concourse import bacc, mybir
    from concourse.bass import ts

    f32 = mybir.dt.float32
    f16 = mybir.dt.float16
    relu = mybir.ActivationFunctionType.Relu

    nc = bacc.Bacc("TRN2", target_bir_lowering=False, debug=False,
                   num_devices=NCORES)
    xa = nc.dram_tensor("xt", [CT, 128, T_SEQ], f16, kind="ExternalInput").ap()
    aa = nc.dram_tensor("at", [JT, 128, N_EMBD], f16, kind="ExternalInput").ap()
    ba = nc.dram_tensor("bt", [8, 128, 4 * N_EMBD], f16,
                        kind="ExternalInput").ap()
    # output is Y^T tiles: [t-half, c-tile, 128 c, 512 t]; the final half
    # group (th=1, c8=7, t 256:512) is emitted separately in fp16.
    ya = nc.dram_tensor("y", [2, 8, 128, 512], f32, kind="ExternalOutput").ap()
    y2a = nc.dram_tensor("y2", [128, 256], f16, kind="ExternalOutput").ap()

    with tile.TileContext(nc) as tc:
        with (
            tc.tile_pool(name="xt", bufs=CT) as xt_pool,
            tc.tile_pool(name="wm", bufs=1) as wm_pool,
            tc.tile_pool(name="ht", bufs=JT) as ht_pool,
            tc.tile_pool(name="at", bufs=6) as at_pool,
            tc.tile_pool(name="bt", bufs=8) as bt_pool,
            tc.tile_pool(name="yo", bufs=4) as yo_pool,
            tc.tile_pool(name="yz", bufs=2) as yz_pool,
            tc.tile_pool(name="ps", bufs=8, space="PSUM") as ps_pool,
        ):
            # A-tile stream all on the sync HWDGE queue (sustains ~135 GB/s,
            # ahead of the 3.46us/tile consumption); at0/at1 first.
            att = {}

            def at_dma(jt):
                t = at_pool.tile([128, N_EMBD], f16, tag="at", name=f"a{jt}")
                nc.sync.dma_start(out=t[:], in_=aa[jt])
                att[jt] = t

            at_dma(0)
            at_dma(1)

            # x tiles [128, 1024]: 0/1/3/5 on the scalar HWDGE queue, 2/4 on
            # gpsimd, 6/7 as the sync queue's 3rd/4th transfers (the sync
            # queue idles after at0/at1 — carrying the two latest-needed x
            # tiles there compresses the arrival curve by ~2us).
            xts = [xt_pool.tile([128, T_SEQ], f16, tag="xt", name=f"x{ct}")
                   for ct in range(CT)]
            for ct in (0, 1, 3, 5):
                nc.scalar.dma_start(out=xts[ct][:], in_=xa[ct])
            for ct in (2, 4):
                nc.gpsimd.dma_start(out=xts[ct][:], in_=xa[ct])
            nc.sync.dma_start(out=xts[6][:], in_=xa[6])
            nc.sync.dma_start(out=xts[7][:], in_=xa[7])

            # at2 rides the sync queue right after xt6/xt7.
            with tc.tile_wait_until(0.0145):
                at_dma(2)

            # PE warmup on a zeroed tile: continuous busy through the input
            # DMA wait so the HAM clock gate opens before the real stream.
            wt = wm_pool.tile([128, 512], f16, tag="warm", name="warm")
            nc.vector.memset(wt[:], 0)
            wps = ps_pool.tile([128, 512], f32, tag="ps", name="wps")
            for i in range(N_WARM):
                nc.tensor.matmul(wps[:], lhsT=wt[:, ts(0, 128)], rhs=wt[:],
                                 start=True, stop=True)

            # phase 1 head: j-tiles 0/1 emitted in input-arrival order
            # (PSUM accumulation over ct commutes; start/stop per first/last
            # emitted ct of each (jt, th) group). Monotone wait stamps pin
            # the Tile scheduler to this order (its own DMA-arrival model
            # otherwise reorders the stream and hoists late-arriving work
            # ahead of ready work).
            head_ps = [[ps_pool.tile([128, 512], f32, tag="ps",
                                     name=f"hps{jt}_{i}") for i in range(2)]
                       for jt in range(HEAD_JT)]
            seen = [0] * HEAD_JT
            head_end = 0.0190
            for k, (jt, ct) in enumerate(_HEAD_ORDER):
                seen[jt] += 1
                with tc.tile_wait_until(0.0120 + 0.00043 * k):
                    for th in range(2):
                        nc.tensor.matmul(
                            head_ps[jt][th][:], lhsT=att[jt][:, ts(ct, 128)],
                            rhs=xts[ct][:, ts(th, 512)],
                            start=(seen[jt] == 1), stop=(seen[jt] == CT))
            hts = []
            for jt in range(HEAD_JT):
                htt = ht_pool.tile([128, T_SEQ], f16, tag="ht", name=f"h{jt}")
                with tc.tile_wait_until(head_end):
                    for th in range(2):
                        nc.scalar.activation(htt[:, ts(th, 512)],
                                             head_ps[jt][th][:], relu)
                hts.append(htt)

            # phase 1 main loop: stationary A-tile per jt, 16 matmuls.
            # Same wait-stamp trick: consumption-schedule stamps keep the
            # at-DMA stream behind xt7 on the sync queue and stop main-loop
            # matmuls from hoisting into the head.
            for jt in range(HEAD_JT, JT):
                with tc.tile_wait_until(
                        head_end + 0.00346 * (jt - HEAD_JT)):
                    if jt > 2:
                        # jt == 2 already streamed in at the 0.0145 stamp;
                        # re-issuing it here (the old behavior) burned 256KB
                        # of critical-window HBM bandwidth on a duplicate.
                        at_dma(jt)
                    a = att[jt]
                    ps = [ps_pool.tile([128, 512], f32, tag="ps",
                                       name=f"ps{jt}_{i}") for i in range(2)]
                    # th-outer: consecutive matmuls accumulate into the SAME
                    # PSUM bank (like phase 2, which measures 512 cyc/MM flat
                    # vs 519.6 for the bank-alternating th-inner order).
                    for th in range(2):
                        for ct in range(CT):
                            nc.tensor.matmul(
                                ps[th][:], lhsT=a[:, ts(ct, 128)],
                                rhs=xts[ct][:, ts(th, 512)],
                                start=(ct == 0), stop=(ct == CT - 1))
                    htt = ht_pool.tile([128, T_SEQ], f16, tag="ht",
                                       name=f"h{jt}")
                    for th in range(2):
                        nc.scalar.activation(htt[:, ts(th, 512)], ps[th][:],
                                             relu)
                    hts.append(htt)

            # B tiles: 1MB DMAs on gpsimd only, runtime-gated behind the x
            # stream. The 1-element gpsimd copy reading xts[7] blocks the
            # gpsimd engine until the last x tile has landed, so the bt
            # transfers cannot eat HBM bandwidth during the head (observed:
            # ungated bt transfers delayed xt5/xt7 to 23/28us and the head
            # stalls re-throttled the PE clock for 13.7us). tile_wait_until
            # additionally stops the scheduler from hoisting them.
            bts = []
            for g in range(8):
                btt = bt_pool.tile([128, 4 * N_EMBD], f16, tag="bt",
                                   name=f"b{g}")
                with tc.tile_wait_until(0.020 + 0.005 * g):
                    nc.gpsimd.tensor_copy(btt[:, 0:1], xts[7][:, 0:1])
                    nc.gpsimd.dma_start(out=btt[:], in_=ba[g])
                bts.append(btt)

            def bt_ap(jt, c8):
                return bts[jt // 4][:, (jt % 4) * N_EMBD + 128 * c8:
                                    (jt % 4) * N_EMBD + 128 * (c8 + 1)]

            # phase 2: Y^T[c,t] accumulated over j. c8-outer/jt-inner so all
            # copies/output DMAs except the last hide under the MM stream.
            for th in range(2):
                for c8 in range(8):
                    if th == 1 and c8 == 7:
                        break
                    pst = ps_pool.tile([128, 512], f32, tag="ps",
                                       name=f"yps{th}_{c8}")
                    for jt in range(JT):
                        nc.tensor.matmul(
                            pst[:], lhsT=bt_ap(jt, c8),
                            rhs=hts[jt][:, ts(th, 512)],
                            start=(jt == 0), stop=(jt == JT - 1))
                    yo = yo_pool.tile([128, 512], f32, tag="yo",
                                      name=f"y{th}_{c8}")
                    if c8 % 2 == 0:
                        nc.vector.tensor_copy(yo[:], pst[:])
                        nc.sync.dma_start(out=ya[th, c8], in_=yo[:])
                    else:
                        nc.scalar.copy(yo[:], pst[:])
                        nc.scalar.dma_start(out=ya[th, c8], in_=yo[:])

            # final group (th=1, c8=7) split into 2x N=256 so the exposed
            # tail is one small fp16 transfer instead of a 256KB fp32 one.
            psA = ps_pool.tile([128, 256], f32, tag="ps", name="ypsA")
            for jt in range(JT):
                nc.tensor.matmul(psA[:], lhsT=bt_ap(jt, 7),
                                 rhs=hts[jt][:, 512:768],
                                 start=(jt == 0), stop=(jt == JT - 1))
            yoA = yz_pool.tile([128, 256], f32, tag="yz", name="yA")
            nc.vector.tensor_copy(yoA[:], psA[:])
            nc.sync.dma_start(out=ya[1, 7, :, 0:256], in_=yoA[:])

            # last two sub-groups at N=128: sub-group 0's copy+DMA hide
            # under sub-group 1's 32 matmuls, so the exposed tail is a
            # single [128,128] fp16 copy + 32KB DMA.
            for half, (t0, t1) in enumerate(((768, 896), (896, 1024))):
                psB = ps_pool.tile([128, 128], f32, tag="ps",
                                   name=f"ypsB{half}")
                for jt in range(JT):
                    nc.tensor.matmul(psB[:], lhsT=bt_ap(jt, 7),
                                     rhs=hts[jt][:, t0:t1],
                                     start=(jt == 0), stop=(jt == JT - 1))
                yoB = yz_pool.tile([128, 128], f16, tag="yzB",
                                   name=f"yB{half}")
                # DVE copy: 2x rate at 16-bit output (~200ns vs scalar's
                # 473ns) on the copies whose latency is tail-exposed.
                nc.vector.tensor_copy(yoB[:], psB[:])
                nc.scalar.dma_start(out=y2a[:, ts(half, 128)], in_=yoB[:])

    nc.compile()
    return nc


class _Runner:
    """Persistent jitted SPMD dispatcher (mirrors bass2jax.run_bass_via_pjrt's
    multi-core branch, but reuses one jax.jit across calls)."""

    def __init__(self, nc):
        _ensure_concourse()
        import jax
        import concourse.mybir as mybir
        from concourse import bass2jax
        from jax.experimental.shard_map import shard_map
        from jax.sharding import Mesh, PartitionSpec

        bass2jax.install_neuronx_cc_hook()
        self.nc = nc
        partition_name = (nc.partition_id_tensor.name
                          if nc.partition_id_tensor else None)
        in_names, out_names, out_avals, zero_shapes = [], [], [], []
        for alloc in nc.m.functions[0].allocations:
            if not isinstance(alloc, mybir.MemoryLocationSet):
                continue
            name = alloc.memorylocations[0].name
            if alloc.kind == "ExternalInput":
                if name != partition_name:
                    in_names.append(name)
            elif alloc.kind == "ExternalOutput":
                shape = tuple(alloc.tensor_shape)
                dtype = mybir.dt.np(alloc.dtype)
                out_names.append(name)
                out_avals.append(jax.core.ShapedArray(shape, dtype))
                zero_shapes.append((shape, dtype))
        self.in_names = list(in_names)
        self.out_names = out_names
        self.out_avals = out_avals
        self.zero_shapes = zero_shapes
        n_params = len(in_names)
        all_in_names = in_names + out_names
        if partition_name is not None:
            all_in_names.append(partition_name)

        def _body(*args):
            operands = list(args)
            if partition_name is not None:
                operands.append(bass2jax.partition_id_tensor())
            outs = bass2jax._bass_exec_p.bind(
                *operands,
                out_avals=tuple(out_avals),
                in_names=tuple(all_in_names),
                out_names=tuple(out_names),
                lowering_input_output_aliases=(),
                sim_require_finite=True,
                sim_require_nnan=True,
                nc=nc,
            )
            return tuple(outs)

        devices = jax.devices()[:NCORES]
        assert len(devices) == NCORES
        self.mesh = Mesh(np.asarray(devices), ("core",))
        n_outs = len(out_names)
        in_specs = (PartitionSpec("core"),) * (n_params + n_outs)
        out_specs = (PartitionSpec("core"),) * n_outs
        self.donate = tuple(range(n_params, n_params + n_outs))
        self.fn = jax.jit(
            shard_map(_body, mesh=self.mesh, in_specs=in_specs,
                      out_specs=out_specs, check_rep=False),
            donate_argnums=self.donate, keep_unused=True)

    def concat_inputs(self, in_maps):
        return [np.concatenate([np.asarray(m[n]) for m in in_maps], axis=0)
                for n in self.in_names]

    def zero_outs(self):
        return [np.zeros((NCORES * s[0], *s[1:]), d)
                for (s, d) in self.zero_shapes]

    def __call__(self, in_maps):
        concat_in = self.concat_inputs(in_maps)
        out_arrs = self.fn(*concat_in, *self.zero_outs())
        return [
            {n: np.asarray(out_arrs[i]).reshape(NCORES, *self.out_avals[i].shape)[c]
             for i, n in enumerate(self.out_names)}
            for c in range(NCORES)
        ]


def _get_runner():
    if "runner" not in _STATE:
        nc = _build_nc()
        _STATE["nc"] = nc
        _STATE["runner"] = _Runner(nc)
    return _STATE["runner"]


# ------------------------------------------------------------------ host side

def _tile_A(A):
    """(4096, 1024) row-gathered Wm -> fp16 'at' tiles [jt, ci, ct*128+jj]."""
    return np.ascontiguousarray(
        A.reshape(JT, 128, CT, 128).transpose(0, 3, 2, 1)).reshape(
            JT, 128, N_EMBD).astype(np.float16)


def _tile_B(Bm):
    """(4096, 1024) row-gathered Wp^T -> fp16 'bt' layout [8, jj, 4jt*c]."""
    t = np.ascontiguousarray(Bm).reshape(8, 4, 128, N_EMBD).astype(np.float16)
    return np.ascontiguousarray(t.transpose(0, 2, 1, 3)).reshape(
        8, 128, 4 * N_EMBD)


def _tile_X(xc):
    """(B, 1024 t, 1024 c) -> per-core fp16 xt tiles [B, ct, ci, t]."""
    return np.ascontiguousarray(xc.transpose(0, 2, 1)).reshape(
        xc.shape[0], CT, 128, T_SEQ).astype(np.float16)


def _untile_Y(res):
    """{'y': [th,c8,ci,tt] f32, 'y2': [ci,256] f16} -> f32 (1024 t, 1024 c)."""
    y = np.ascontiguousarray(
        res["y"].transpose(0, 3, 1, 2)).reshape(T_SEQ, N_EMBD).astype(
            np.float32, copy=False)
    y[768:1024, 896:1024] = res["y2"].T.astype(np.float32)
    return y


def _device_forward(xc, at_list, bt_list):
    """y[b] = relu(xc[b] @ A^T) @ B for 8 cores at once."""
    xts = _tile_X(xc)
    in_maps = []
    for b in range(NCORES):
        in_maps.append({"xt": xts[b], "at": at_list[b], "bt": bt_list[b]})
    try:
        results = _get_runner()(in_maps)
    except Exception:
        # fall back to the supported dispatch path (fresh jit per call)
        from concourse.bass_utils import run_bass_kernel_spmd
        if "nc" not in _STATE:
            _STATE["nc"] = _build_nc()
        results = run_bass_kernel_spmd(
            _STATE["nc"], in_maps, list(range(NCORES))).results
    return np.stack([_untile_Y(results[b]) for b in range(NCORES)])


def _topk_mask(ck, k):
    # matches jax.lax.top_k tie-breaking (first index wins) via stable argsort
    order = np.argsort(-ck, axis=1, kind="stable")[:, :k]
    mask = np.zeros_like(ck)
    np.put_along_axis(mask, order, 1.0, axis=1)
    return mask


def kernel(x, Wm, Wp, Wc1, Wc2):
    x = np.ascontiguousarray(np.asarray(x, dtype=np.float32))
    Wm = np.ascontiguousarray(np.asarray(Wm, dtype=np.float32))
    Wp = np.ascontiguousarray(np.asarray(Wp, dtype=np.float32))
    Wc1 = np.asarray(Wc1, dtype=np.float32)
    Wc2 = np.asarray(Wc2, dtype=np.float32)
    B = x.shape[0]
    assert B == NCORES and x.shape[1] == T_SEQ and x.shape[2] == N_EMBD

    WpT = None  # lazily built; only needed on non-base iterations
    base = np.zeros((B, FULL), np.float32)
    base[:, :ITER] = 1.0

    xc, active, history, done = x, base, base.copy(), False
    for _ in range(3):
        if done:
            break
        idxs = [np.flatnonzero(active[b]) for b in range(B)]
        is_base = all(ix.shape[0] == ITER and ix[0] == 0 and ix[-1] == ITER - 1
                      for ix in idxs) and all(
                          np.array_equal(ix, idxs[0]) for ix in idxs[1:])
        if is_base and np.array_equal(idxs[0], np.arange(ITER)):
            at = _tile_A(Wm[:ITER])
            bt = _tile_B(np.ascontiguousarray(Wp[:, :ITER].T))
            at_list = [at] * B
            bt_list = [bt] * B
        else:
            if WpT is None:
                WpT = np.ascontiguousarray(Wp.T)
            at_list = [_tile_A(np.ascontiguousarray(Wm[ix])) for ix in idxs]
            bt_list = [_tile_B(WpT[ix]) for ix in idxs]

        y = _device_forward(xc, at_list, bt_list)

        pooled = y.mean(axis=1)
        ck = np.maximum(pooled @ Wc1.T, 0.0) @ Wc2.T
        new_mask = _topk_mask(ck, ITER)
        combined = np.clip(history + new_mask, 0.0, 1.0)
        novelty = (combined - history).sum(axis=1).mean() / ITER
        xc, active, history = y, new_mask, combined
        done = bool(novelty < R_NOVELTY)

    return xc.astype(np.float32, copy=False)



# revision 4
# speedup vs baseline: 1.0142x; 1.0016x over previous
"""Trainium2 Bass kernel for nn_LoopedMLP (moe_routing).

Reference semantics (B=8, T=1024, C=1024, ITER=4096, FULL=12288, R=0.7):
a 3-step scan; each step computes
    y = relu((x @ Wm^T) * active_mask) @ Wp^T
then a control net on mean-pooled y picks a new top-4096 column mask, and a
batch-mean "novelty" below R freezes the state for remaining steps.

Because relu((x@Wm^T)*mask) @ Wp^T only touches the masked 4096 columns, each
step is exactly a dense per-sample MLP over the *gathered* active columns:
    y = relu(x @ A^T) @ B,   A = Wm[idx], B = Wp[:, idx]^T,  |idx| = 4096.

Strategy: data-parallel over B (1 sample per NeuronCore, 8 cores). The device
kernel computes the gathered MLP; all routing logic (control net, top-k,
novelty, done) runs on host between launches. With random weights the step-1
novelty is ~2/3 < R, so in practice exactly one device launch happens.

Device compute per core: two back-to-back matmul chains in fp16 (stationary
weights, 1 moving column/cycle at N=512) with fp32 PSUM accumulation:
  phase 1: H^T[j,t]   = relu( sum_c A^T[c,j] * X^T[c,t] )  (j=4096, t=1024, c=1024)
  phase 2: Y^T[c,t]   = sum_j B[j,c] * H^T[j,t]
~1024 matmuls of [128x128]@[128x512] at ~216 ns warm => ~221 us of PE stream.
(fp8 DoubleRow would be 2x but its e6m3 operand upcast caps precision at 3
mantissa bits => >=4e-2 abs-max rel error per fp8 phase vs the 2e-2 gate,
verified numerically — fp16 is the fastest dtype that passes.)

Startup/tail engineering vs the 242.2 us baseline (all trace-driven;
measured best 237.9 us, stall-free stream, occasional chip-level P0 power
throttling can still stretch a run to ~285 us — outside kernel control):
 - input arrival curve: per-queue DMA sustains only ~90-135 GB/s
   (3 queues share ~270 GB/s of HBM), so the 2.5 MB the head needs defines
   a critical stream-start time S* ~= 12 us. x tiles ride scalar (0/1/3/5)
   and gpsimd (2/4); the sync queue carries at0, at1 then the two
   latest-needed x tiles (6/7) before the at stream — this balances the
   three queues and compresses S* by ~2 us vs a 2-queue x split.
 - 11 warmup matmuls on a zeroed tile keep the PE continuously busy from
   ~8 us so the HAM clock gate opens at ~11.5 us (any pre-flip idle gap
   restarts its 3.4 us activity window — measured) and the real stream
   starts at S* fully warm; the head (j-tiles 0/1, PSUM ct-accumulation
   commutes) is emitted in measured-arrival order, giving a zero-stall
   stream end to end.
 - the Tile scheduler orders each engine queue from its own cost model and
   otherwise hoists late work ahead of ready work (observed on 3 queues);
   monotone tc.tile_wait_until stamps pin every queue to the planned
   order, and the 8 MB b-tile prefetch is runtime-gated behind the x
   stream by a 1-element gpsimd copy that reads xts[7].
 - phase-2 final accumulation group (th=1, c8=7) split into N=256 +
   2x N=128 groups; every copy/DMA except the last 32KB fp16 one hides
   under the following sub-group's matmuls, minimizing the exposed tail.

Session-2 trace findings (exec ~238.3us fresh-run, +-1.2us thermal drift):
 - the stream is at its floor: ~8.0us fixed preamble (3.3us engine-init
   sem wait + ~1.25us iram TENSOR_LOAD + barriers), 12 warmups bridging
   to the x0 arrival (visible ~12.0us, first real MM 12.08 -- zero slack),
   then a ZERO-GAP TensorMatrix stream to ~233.6us, ~4.7us fixed tail
   (cast+DMA chain ~1.3 + completion sems ~0.9 + teardown ~2.5).
 - the arrival window is DMA-parallelism-bound at ~280GB/s aggregate
   (per-queue ~60-90GB/s, descriptor-latency-limited). Removing the
   duplicate at2 DMA (a jt==2 re-issue in the main loop) freed 256KB;
   the scheduler backfills with the at stream, so last-x stays ~17.0us.
 - measured dead ends: explicitly throttling the at stream behind x
   (FEWER active queues => aggregate drops to ~236GB/s, exec +4us; a
   mild variant gating only a4+, which have 6us+ of slack, still lost
   every interleaved A/B pair by 0.5-1.0us -- the DMA engines genuinely
   go faster with more concurrent transfers, so do NOT retry gating);
   splitting first/last tiles into 128KB halves (descriptor-bound: a
   half takes ~2.6-4.3us vs 3.2 for a full tile, and a real MM that
   stalls pre-HAM-flip restarts the 3.4us clock-gate window, exec +1.2);
   hw-loop compression of phase 2 (branch overhead inside the dense MM
   stream, not attempted); fp8 (e6m3 upcast, fails the 2e-2 gate).
"""

import os
import sys

import numpy as np


def _ensure_concourse():
    try:
        import concourse  # noqa: F401
    except ImportError:
        for p in ("/opt/trn_rl_repo", "/root/.axon_site/_ro/trn_rl_repo"):
            if os.path.isdir(p) and p not in sys.path:
                sys.path.insert(0, p)
        import concourse  # noqa: F401


N_EMBD = 1024
T_SEQ = 1024
ITER = 4096
FULL = 12288
R_NOVELTY = 0.7
NCORES = 8
JT = ITER // 128   # 32 j-tiles
CT = N_EMBD // 128  # 8 c-tiles
HEAD_JT = 2        # j-tiles computed ct-outer while the x stream lands
N_WARM = 12        # 8 cold + 4 warm matmuls: continuous PE busy from ~8.0us
                   # flips the HAM clock gate at ~11.4us and lands exactly at
                   # the critical stream-start time S* ~= 12.0us, after which
                   # the measured input-arrival curve stays ahead of the
                   # 433ns/pair consumption with zero stalls (a stall before
                   # the flip restarts the 3.4us activity window — measured).

# head emission order (jt, ct): greedy vs measured arrivals for the queue
# assignment below (xt0,1,3,5 on scalar; xt2,4 on gpsimd; sync runs
# at0, at1, xt6, xt7, then the at stream).
_HEAD_ORDER = [(0, 0), (0, 2), (1, 0), (1, 2), (0, 1), (1, 1),
               (0, 4), (1, 4), (0, 6), (1, 6), (0, 3), (1, 3),
               (0, 7), (1, 7), (0, 5), (1, 5)]

_STATE: dict = {}


# ---------------------------------------------------------------- device side

def _build_nc():
    _ensure_concourse()
    import concourse.tile as tile
    from concourse import bacc, mybir
    from concourse.bass import ts

    f32 = mybir.dt.float32
    f16 = mybir.dt.float16
    relu = mybir.ActivationFunctionType.Relu

    nc = bacc.Bacc("TRN2", target_bir_lowering=False, debug=False,
                   num_devices=NCORES)
    xa = nc.dram_tensor("xt", [CT, 128, T_SEQ], f16, kind="ExternalInput").ap()
    aa = nc.dram_tensor("at", [JT, 128, N_EMBD], f16, kind="ExternalInput").ap()
    ba = nc.dram_tensor("bt", [8, 128, 4 * N_EMBD], f16,
                        kind="ExternalInput").ap()
    # output is Y^T tiles: [t-half, c-tile, 128 c, 512 t]; the final half
    # group (th=1, c8=7, t 256:512) is emitted separately in fp16.
    ya = nc.dram_tensor("y", [2, 8, 128, 512], f32, kind="ExternalOutput").ap()
    y2a = nc.dram_tensor("y2", [128, 256], f16, kind="ExternalOutput").ap()

    with tile.TileContext(nc) as tc:
        with (
            tc.tile_pool(name="xt", bufs=CT) as xt_pool,
            tc.tile_pool(name="wm", bufs=1) as wm_pool,
            tc.tile_pool(name="ht", bufs=JT) as ht_pool,
            tc.tile_pool(name="at", bufs=6) as at_pool,
            tc.tile_pool(name="bt", bufs=8) as bt_pool,
            tc.tile_pool(name="yo", bufs=4) as yo_pool,
            tc.tile_pool(name="yz", bufs=2) as yz_pool,
            tc.tile_pool(name="ps", bufs=8, space="PSUM") as ps_pool,
        ):
            # A-tile stream all on the sync HWDGE queue (sustains ~135 GB/s,
            # ahead of the 3.46us/tile consumption); at0/at1 first.
            att = {}

            def at_dma(jt):
                t = at_pool.tile([128, N_EMBD], f16, tag="at", name=f"a{jt}")
                nc.sync.dma_start(out=t[:], in_=aa[jt])
                att[jt] = t

            at_dma(0)
            at_dma(1)

            # x tiles [128, 1024]: 0/1/3/5 on the scalar HWDGE queue, 2/4 on
            # gpsimd, 6/7 as the sync queue's 3rd/4th transfers (the sync
            # queue idles after at0/at1 — carrying the two latest-needed x
            # tiles there compresses the arrival curve by ~2us).
            xts = [xt_pool.tile([128, T_SEQ], f16, tag="xt", name=f"x{ct}")
                   for ct in range(CT)]
            for ct in (0, 1, 3, 5):
                nc.scalar.dma_start(out=xts[ct][:], in_=xa[ct])
            for ct in (2, 4):
                nc.gpsimd.dma_start(out=xts[ct][:], in_=xa[ct])
            nc.sync.dma_start(out=xts[6][:], in_=xa[6])
            nc.sync.dma_start(out=xts[7][:], in_=xa[7])

            # at2 rides the sync queue right after xt6/xt7.
            with tc.tile_wait_until(0.0145):
                at_dma(2)

            # PE warmup on a zeroed tile: continuous busy through the input
            # DMA wait so the HAM clock gate opens before the real stream.
            wt = wm_pool.tile([128, 512], f16, tag="warm", name="warm")
            nc.vector.memset(wt[:], 0)
            wps = ps_pool.tile([128, 512], f32, tag="ps", name="wps")
            for i in range(N_WARM):
                nc.tensor.matmul(wps[:], lhsT=wt[:, ts(0, 128)], rhs=wt[:],
                                 start=True, stop=True)

            # phase 1 head: j-tiles 0/1 emitted in input-arrival order
            # (PSUM accumulation over ct commutes; start/stop per first/last
            # emitted ct of each (jt, th) group). Monotone wait stamps pin
            # the Tile scheduler to this order (its own DMA-arrival model
            # otherwise reorders the stream and hoists late-arriving work
            # ahead of ready work).
            head_ps = [[ps_pool.tile([128, 512], f32, tag="ps",
                                     name=f"hps{jt}_{i}") for i in range(2)]
                       for jt in range(HEAD_JT)]
            seen = [0] * HEAD_JT
            head_end = 0.0190
            for k, (jt, ct) in enumerate(_HEAD_ORDER):
                seen[jt] += 1
                with tc.tile_wait_until(0.0120 + 0.00043 * k):
                    for th in range(2):
                        nc.tensor.matmul(
                            head_ps[jt][th][:], lhsT=att[jt][:, ts(ct, 128)],
                            rhs=xts[ct][:, ts(th, 512)],
                            start=(seen[jt] == 1), stop=(seen[jt] == CT))
            hts = []
            for jt in range(HEAD_JT):
                htt = ht_pool.tile([128, T_SEQ], f16, tag="ht", name=f"h{jt}")
                with tc.tile_wait_until(head_end):
                    for th in range(2):
                        nc.scalar.activation(htt[:, ts(th, 512)],
                                             head_ps[jt][th][:], relu)
                hts.append(htt)

            # phase 1 main loop: stationary A-tile per jt, 16 matmuls.
            # Same wait-stamp trick: consumption-schedule stamps keep the
            # at-DMA stream behind xt7 on the sync queue and stop main-loop
            # matmuls from hoisting into the head.
            for jt in range(HEAD_JT, JT):
                with tc.tile_wait_until(
                        head_end + 0.00346 * (jt - HEAD_JT)):
                    if jt > 2:
                        # jt == 2 already streamed in at the 0.0145 stamp;
                        # re-issuing it here (the old behavior) burned 256KB
                        # of critical-window HBM bandwidth on a duplicate.
                        at_dma(jt)
                    a = att[jt]
                    ps = [ps_pool.tile([128, 512], f32, tag="ps",
                                       name=f"ps{jt}_{i}") for i in range(2)]
                    # th-outer: consecutive matmuls accumulate into the SAME
                    # PSUM bank (like phase 2, which measures 512 cyc/MM flat
                    # vs 519.6 for the bank-alternating th-inner order).
                    for th in range(2):
                        for ct in range(CT):
                            nc.tensor.matmul(
                                ps[th][:], lhsT=a[:, ts(ct, 128)],
                                rhs=xts[ct][:, ts(th, 512)],
                                start=(ct == 0), stop=(ct == CT - 1))
                    htt = ht_pool.tile([128, T_SEQ], f16, tag="ht",
                                       name=f"h{jt}")
                    for th in range(2):
                        nc.scalar.activation(htt[:, ts(th, 512)], ps[th][:],
                                             relu)
                    hts.append(htt)

            # B tiles: 1MB DMAs on gpsimd only, runtime-gated behind the x
            # stream. The 1-element gpsimd copy reading xts[7] blocks the
            # gpsimd engine until the last x tile has landed, so the bt
            # transfers cannot eat HBM bandwidth during the head (observed:
            # ungated bt transfers delayed xt5/xt7 to 23/28us and the head
            # stalls re-throttled the PE clock for 13.7us). tile_wait_until
            # additionally stops the scheduler from hoisting them.
            bts = []
            for g in range(8):
                btt = bt_pool.tile([128, 4 * N_EMBD], f16, tag="bt",
                                   name=f"b{g}")
                with tc.tile_wait_until(0.020 + 0.005 * g):
                    nc.gpsimd.tensor_copy(btt[:, 0:1], xts[7][:, 0:1])
                    nc.gpsimd.dma_start(out=btt[:], in_=ba[g])
                bts.append(btt)

            def bt_ap(jt, c8):
                return bts[jt // 4][:, (jt % 4) * N_EMBD + 128 * c8:
                                    (jt % 4) * N_EMBD + 128 * (c8 + 1)]

            # phase 2: Y^T[c,t] accumulated over j. c8-outer/jt-inner so all
            # copies/output DMAs except the last hide under the MM stream.
            for th in range(2):
                for c8 in range(8):
                    if th == 1 and c8 == 7:
                        break
                    pst = ps_pool.tile([128, 512], f32, tag="ps",
                                       name=f"yps{th}_{c8}")
                    for jt in range(JT):
                        nc.tensor.matmul(
                            pst[:], lhsT=bt_ap(jt, c8),
                            rhs=hts[jt][:, ts(th, 512)],
                            start=(jt == 0), stop=(jt == JT - 1))
                    yo = yo_pool.tile([128, 512], f32, tag="yo",
                                      name=f"y{th}_{c8}")
                    if c8 % 2 == 0:
                        nc.vector.tensor_copy(yo[:], pst[:])
                        nc.sync.dma_start(out=ya[th, c8], in_=yo[:])
                    else:
                        nc.scalar.copy(yo[:], pst[:])
                        nc.scalar.dma_start(out=ya[th, c8], in_=yo[:])

            # final group (th=1, c8=7) split into 2x N=256 so the exposed
            # tail is one small fp16 transfer instead of a 256KB fp32 one.
            psA = ps_pool.tile([128, 256], f32, tag="ps", name="ypsA")
            for jt in range(JT):
                nc.tensor.matmul(psA[:], lhsT=bt_ap(jt, 7),
                                 rhs=hts[jt][:, 512:768],
                                 start=(jt == 0), stop=(jt == JT - 1))
            yoA = yz_pool.tile([128, 256], f32, tag="yz", name="yA")
            nc.vector.tensor_copy(yoA[:], psA[:])
            nc.sync.dma_start(out=ya[1, 7, :, 0:256], in_=yoA[:])

            # last two sub-groups at N=128: sub-group 0's copy+DMA hide
            # under sub-group 1's 32 matmuls, so the exposed tail is a
            # single [128,128] fp16 copy + 32KB DMA.
            for half, (t0, t1) in enumerate(((768, 896), (896, 1024))):
                psB = ps_pool.tile([128, 128], f32, tag="ps",
                                   name=f"ypsB{half}")
                for jt in range(JT):
                    nc.tensor.matmul(psB[:], lhsT=bt_ap(jt, 7),
                                     rhs=hts[jt][:, t0:t1],
                                     start=(jt == 0), stop=(jt == JT - 1))
                yoB = yz_pool.tile([128, 128], f16, tag="yzB",
                                   name=f"yB{half}")
                # DVE copy: 2x rate at 16-bit output (~200ns vs scalar's
                # 473ns) on the copies whose latency is tail-exposed.
                nc.vector.tensor_copy(yoB[:], psB[:])
                nc.scalar.dma_start(out=y2a[:, ts(half, 128)], in_=yoB[:])

    nc.compile()
    return nc


class _Runner:
    """Persistent jitted SPMD dispatcher (mirrors bass2jax.run_bass_via_pjrt's
    multi-core branch, but reuses one jax.jit across calls)."""

    def __init__(self, nc):
        _ensure_concourse()
        import jax
        import concourse.mybir as mybir
        from concourse import bass2jax
        from jax.experimental.shard_map import shard_map
        from jax.sharding import Mesh, PartitionSpec

        bass2jax.install_neuronx_cc_hook()
        self.nc = nc
        partition_name = (nc.partition_id_tensor.name
                          if nc.partition_id_tensor else None)
        in_names, out_names, out_avals, zero_shapes = [], [], [], []
        for alloc in nc.m.functions[0].allocations:
            if not isinstance(alloc, mybir.MemoryLocationSet):
                continue
            name = alloc.memorylocations[0].name
            if alloc.kind == "ExternalInput":
                if name != partition_name:
                    in_names.append(name)
            elif alloc.kind == "ExternalOutput":
                shape = tuple(alloc.tensor_shape)
                dtype = mybir.dt.np(alloc.dtype)
                out_names.append(name)
                out_avals.append(jax.core.ShapedArray(shape, dtype))
                zero_shapes.append((shape, dtype))
        self.in_names = list(in_names)
        self.out_names = out_names
        self.out_avals = out_avals
        self.zero_shapes = zero_shapes
        n_params = len(in_names)
        all_in_names = in_names + out_names
        if partition_name is not None:
            all_in_names.append(partition_name)

        def _body(*args):
            operands = list(args)
            if partition_name is not None:
                operands.append(bass2jax.partition_id_tensor())
            outs = bass2jax._bass_exec_p.bind(
                *operands,
                out_avals=tuple(out_avals),
                in_names=tuple(all_in_names),
                out_names=tuple(out_names),
                lowering_input_output_aliases=(),
                sim_require_finite=True,
                sim_require_nnan=True,
                nc=nc,
            )
            return tuple(outs)

        devices = jax.devices()[:NCORES]
        assert len(devices) == NCORES
        self.mesh = Mesh(np.asarray(devices), ("core",))
        n_outs = len(out_names)
        in_specs = (PartitionSpec("core"),) * (n_params + n_outs)
        out_specs = (PartitionSpec("core"),) * n_outs
        self.donate = tuple(range(n_params, n_params + n_outs))
        self.fn = jax.jit(
            shard_map(_body, mesh=self.mesh, in_specs=in_specs,
                      out_specs=out_specs, check_rep=False),
            donate_argnums=self.donate, keep_unused=True)

    def concat_inputs(self, in_maps):
        return [np.concatenate([np.asarray(m[n]) for m in in_maps], axis=0)
                for n in self.in_names]

    def zero_outs(self):
        return [np.zeros((NCORES * s[0], *s[1:]), d)
                for (s, d) in self.zero_shapes]

    def __call__(self, in_maps):
        concat_in = self.concat_inputs(in_maps)
        out_arrs = self.fn(*concat_in, *self.zero_outs())
        return [
            {n: np.asarray(out_arrs[i]).reshape(NCORES, *self.out_avals[i].shape)[c]
             for i, n in enumerate(self.out_names)}
            for c in range(NCORES)
        ]


def _get_runner():
    if "runner" not in _STATE:
        nc = _build_nc()
        _STATE["nc"] = nc
        _STATE["runner"] = _Runner(nc)
    return _STATE["runner"]


# ------------------------------------------------------------------ host side

def _tile_A(A):
    """(4096, 1024) row-gathered Wm -> fp16 'at' tiles [jt, ci, ct*128+jj]."""
    return np.ascontiguousarray(
        A.reshape(JT, 128, CT, 128).transpose(0, 3, 2, 1)).reshape(
            JT, 128, N_EMBD).astype(np.float16)


def _tile_B(Bm):
    """(4096, 1024) row-gathered Wp^T -> fp16 'bt' layout [8, jj, 4jt*c]."""
    t = np.ascontiguousarray(Bm).reshape(8, 4, 128, N_EMBD).astype(np.float16)
    return np.ascontiguousarray(t.transpose(0, 2, 1, 3)).reshape(
        8, 128, 4 * N_EMBD)


def _tile_X(xc):
    """(B, 1024 t, 1024 c) -> per-core fp16 xt tiles [B, ct, ci, t]."""
    return np.ascontiguousarray(xc.transpose(0, 2, 1)).reshape(
        xc.shape[0], CT, 128, T_SEQ).astype(np.float16)


def _untile_Y(res):
    """{'y': [th,c8,ci,tt] f32, 'y2': [ci,256] f16} -> f32 (1024 t, 1024 c)."""
    y = np.ascontiguousarray(
        res["y"].transpose(0, 3, 1, 2)).reshape(T_SEQ, N_EMBD).astype(
            np.float32, copy=False)
    y[768:1024, 896:1024] = res["y2"].T.astype(np.float32)
    return y


def _device_forward(xc, at_list, bt_list):
    """y[b] = relu(xc[b] @ A^T) @ B for 8 cores at once."""
    xts = _tile_X(xc)
    in_maps = []
    for b in range(NCORES):
        in_maps.append({"xt": xts[b], "at": at_list[b], "bt": bt_list[b]})
    try:
        results = _get_runner()(in_maps)
    except Exception:
        # fall back to the supported dispatch path (fresh jit per call)
        from concourse.bass_utils import run_bass_kernel_spmd
        if "nc" not in _STATE:
            _STATE["nc"] = _build_nc()
        results = run_bass_kernel_spmd(
            _STATE["nc"], in_maps, list(range(NCORES))).results
    return np.stack([_untile_Y(results[b]) for b in range(NCORES)])


def _topk_mask(ck, k):
    # matches jax.lax.top_k tie-breaking (first index wins) via stable argsort
    order = np.argsort(-ck, axis=1, kind="stable")[:, :k]
    mask = np.zeros_like(ck)
    np.put_along_axis(mask, order, 1.0, axis=1)
    return mask


def kernel(x, Wm, Wp, Wc1, Wc2):
    x = np.ascontiguousarray(np.asarray(x, dtype=np.float32))
    Wm = np.ascontiguousarray(np.asarray(Wm, dtype=np.float32))
    Wp = np.ascontiguousarray(np.asarray(Wp, dtype=np.float32))
    Wc1 = np.asarray(Wc1, dtype=np.float32)
    Wc2 = np.asarray(Wc2, dtype=np.float32)
    B = x.shape[0]
    assert B == NCORES and x.shape[1] == T_SEQ and x.shape[2] == N_EMBD

    WpT = None  # lazily built; only needed on non-base iterations
    base = np.zeros((B, FULL), np.float32)
    base[:, :ITER] = 1.0

    xc, active, history, done = x, base, base.copy(), False
    for _ in range(3):
        if done:
            break
        idxs = [np.flatnonzero(active[b]) for b in range(B)]
        is_base = all(ix.shape[0] == ITER and ix[0] == 0 and ix[-1] == ITER - 1
                      for ix in idxs) and all(
                          np.array_equal(ix, idxs[0]) for ix in idxs[1:])
        if is_base and np.array_equal(idxs[0], np.arange(ITER)):
            at = _tile_A(Wm[:ITER])
            bt = _tile_B(np.ascontiguousarray(Wp[:, :ITER].T))
            at_list = [at] * B
            bt_list = [bt] * B
        else:
            if WpT is None:
                WpT = np.ascontiguousarray(Wp.T)
            at_list = [_tile_A(np.ascontiguousarray(Wm[ix])) for ix in idxs]
            bt_list = [_tile_B(WpT[ix]) for ix in idxs]

        y = _device_forward(xc, at_list, bt_list)

        pooled = y.mean(axis=1)
        ck = np.maximum(pooled @ Wc1.T, 0.0) @ Wc2.T
        new_mask = _topk_mask(ck, ITER)
        combined = np.clip(history + new_mask, 0.0, 1.0)
        novelty = (combined - history).sum(axis=1).mean() / ITER
        xc, active, history = y, new_mask, combined
        done = bool(novelty < R_NOVELTY)

    return xc.astype(np.float32, copy=False)

